# revision 18
# baseline (speedup 1.0000x reference)
"""GroupedQueryAttention on 8 Trainium2 NeuronCores.

Sharding: core c = 4*b + g handles batch b (of 2) and KV group g (of 4),
i.e. 4 query heads (512 q-dims) + one 128-dim K/V head. o_proj is computed
as per-group partials (transposed layout); partials are reduced with a
per-t-chunk fp16 ReduceScatter across the 4 cores of each batch, pipelined
behind the next chunk's compute so only the last chunk's collective is
exposed. Core c ends up with a [512 d-rows x 512 t] quarter of out^T per
chunk; the host reassembles.

All matmuls run in fp16 (1 PE cycle/row) with fp32 PSUM accumulation.
Layouts avoid transposing the big P matrix:
  - projections produce Q^T/K^T directly (lhsT=W tile, rhs=x^T tile),
    processed panel-major in groups of 3 chains so the PE stays busy
    while x^T panels stream in
  - scores are computed as S^T = (K^T).T @ Q^T
  - exp(S^T) = P^T feeds A@V as lhsT directly
  - V carries an extra ones-column so the softmax denominator falls out
    of the A@V matmul for free; normalization is applied to the small
    A@V output rather than to P.
  - the attention inner loop is software-pipelined: the next (h,s) score
    matmul is issued ahead of the current AV matmuls so the PE stays busy
    while the Scalar engine runs exp; the PE transposes of the finished
    head's output are deferred one iteration so their vector deps resolve
    off the critical path.
  - weight loads are single descriptors on the Scalar DMA queue, x^T
    panel loads on the Sync queue.
"""

import math
import sys

import numpy as np

sys.path.insert(0, "/opt/trn_rl_repo")

B = 2
T = 2048
D = 2048
HEADS = 16
GROUPS = 4
HD = 128  # head dim
M = HEADS // GROUPS  # heads per group = 4
GQ = M * HD  # q dims per group = 512
SCALE = 1.0 / math.sqrt(HD)
N_CORES = 8
TCH = 512  # t chunk
NTCH = T // TCH  # 4
NSB = T // 128  # 16 s blocks
NKS = D // 128  # 16 contraction steps for projections

_COMPILED = {}


def _build():
    import concourse.bass as bass
    import concourse.mybir as mybir
    import concourse.tile as tile
    from concourse import bacc
    from concourse.masks import make_identity

    f16 = mybir.dt.float16
    f32 = mybir.dt.float32
    Exp = mybir.ActivationFunctionType.Exp

    nc = bacc.Bacc("TRN2", target_bir_lowering=False, num_devices=N_CORES)

    xT = nc.declare_dram_parameter("xT", [D, T], f16, isOutput=False)
    # weights host-rearranged to partition-major single-DMA layouts
    wq = nc.declare_dram_parameter("wq", [128, NKS * GQ], f16, isOutput=False)
    wk = nc.declare_dram_parameter("wk", [128, NKS * HD], f16, isOutput=False)
    wv = nc.declare_dram_parameter("wv", [128, NKS * HD], f16, isOutput=False)
    wo = nc.declare_dram_parameter("wo", [128, M * D], f16, isOutput=False)
    bqs_d = nc.declare_dram_parameter("bqs", [128, M], f32, isOutput=False)
    bks_d = nc.declare_dram_parameter("bks", [128, 1], f32, isOutput=False)
    bvs_d = nc.declare_dram_parameter("bvs", [128, 1], f32, isOutput=False)
    bo4_d = nc.declare_dram_parameter("bo4", [128, D // 128], f32, isOutput=False)
    # per-chunk ReduceScatter output: core r of the 4-core group gets rows
    # [512r, 512(r+1)) of out^T for each t-chunk
    outT = nc.declare_dram_parameter(
        "outT", [NTCH, D // 4, TCH], f16, isOutput=True
    )

    groups = [[0, 1, 2, 3], [4, 5, 6, 7]]

    with tile.TileContext(nc) as tc:
        with (
            tc.tile_pool(name="const", bufs=1) as const,
            tc.tile_pool(name="work", bufs=2) as work,
            tc.tile_pool(name="psum", bufs=1, space="PSUM") as psum,
            tc.tile_pool(name="dram", bufs=1, space="DRAM") as dram,
        ):
            ident = const.tile([128, 128], f16)
            make_identity(nc, ident)
            bqs = const.tile([128, M], f32)
            bks = const.tile([128, 1], f32)
            bvs = const.tile([128, 1], f32)
            bo4 = const.tile([128, D // 128], f32)

            # x^T panels as independent tiles so the first chain steps only
            # depend on the panels they actually read
            xt = [const.tile([128, T], f16, name=f"xt{i}") for i in range(NKS)]
            wq_sb = const.tile([128, NKS, GQ], f16)
            wk_sb = const.tile([128, NKS, HD], f16)
            wv_sb = const.tile([128, NKS, HD], f16)
            wo_sb = const.tile([128, M, D], f16)
            # weights on the Scalar DMA queue (single descriptors), x^T
            # panels on the Sync queue in consumption order
            nc.scalar.dma_start(wk_sb[:, :, :], wk[:])
            nc.scalar.dma_start(wv_sb[:, :, :], wv[:])
            nc.scalar.dma_start(bqs[:], bqs_d[:])
            nc.scalar.dma_start(bks[:], bks_d[:])
            nc.scalar.dma_start(bvs[:], bvs_d[:])
            nc.scalar.dma_start(bo4[:], bo4_d[:])
            for i in range(NKS):
                nc.sync.dma_start(xt[i][:], xT[i * 128 : (i + 1) * 128, :])

            # PE warmup: keep the tensor engine continuously busy during the
            # input DMA window so DVFS is fully ramped when real work starts
            for _ in range(120):
                tp = psum.tile([128, 128], f16, tag="tp", bufs=1, name="tp")
                nc.tensor.transpose(tp[:], ident[:], ident[:])

            qt = const.tile([128, M, T], f16)
            kt = const.tile([128, T], f16)
            vt_sb = const.tile([128, T], f16)
            v_sb = const.tile([128, NSB, 132], f16)
            nc.vector.memset(v_sb[:, :, 128:129], 1.0)

            # ---- projections, panel-major in groups of 3 chains ----
            # chain = (weight tile, col-slice, t-chunk, writeback fn)
            def wb_k(acc, tc_i):
                nc.vector.tensor_scalar_add(
                    kt[:, tc_i * TCH : (tc_i + 1) * TCH], acc[:], bks[:, 0:1]
                )

            def wb_v(acc, tc_i):
                nc.vector.tensor_scalar_add(
                    vt_sb[:, tc_i * TCH : (tc_i + 1) * TCH], acc[:], bvs[:, 0:1]
                )

            def wb_q(h):
                def wb(acc, tc_i):
                    nc.vector.tensor_scalar(
                        qt[:, h, tc_i * TCH : (tc_i + 1) * TCH],
                        acc[:],
                        SCALE,
                        bqs[:, h : h + 1],
                        op0=mybir.AluOpType.mult,
                        op1=mybir.AluOpType.add,
                    )

                return wb

            chains = []
            for tc_i in range(NTCH):
                chains.append((wk_sb, 0, HD, tc_i, wb_k))
            for tc_i in range(NTCH):
                chains.append((wv_sb, 0, HD, tc_i, wb_v))
            for tc_i in range(NTCH):
                for h in range(M):
                    chains.append((wq_sb, h * 128, (h + 1) * 128, tc_i, wb_q(h)))

            # big weight loads issued after the K/V chains exist so they
            # don't contend with the early x^T panels for HBM bandwidth
            nc.scalar.dma_start(wq_sb[:, :, :], wq[:])
            nc.scalar.dma_start(wo_sb[:, :, :], wo[:])

            for g0 in range(0, len(chains), 3):
                grp = chains[g0 : g0 + 3]
                accs = [
                    psum.tile([128, TCH], f32, tag="acc", bufs=3, name="acc")
                    for _ in grp
                ]
                for ks in range(NKS):
                    for ci, (wtile, c0, c1, tc_i, _wb) in enumerate(grp):
                        nc.tensor.matmul(
                            accs[ci][:],
                            wtile[:, ks, c0:c1],
                            xt[ks][:, tc_i * TCH : (tc_i + 1) * TCH],
                            start=(ks == 0),
                            stop=(ks == NKS - 1),
                        )
                for ci, (_w, _c0, _c1, tc_i, wb) in enumerate(grp):
                    wb(accs[ci], tc_i)

            # V natural [s, hd] (+ ones col) via PE transpose
            for s in range(NSB):
                tp = psum.tile([128, 128], f16, tag="tp", bufs=1, name="tp")
                nc.tensor.transpose(tp[:], vt_sb[:, s * 128 : (s + 1) * 128], ident[:])
                nc.vector.tensor_copy(v_sb[:, s, 0:128], tp[:])

            # ---- attention + o_proj per t-chunk, RS pipelined behind ----
            def issue_score(h, s, tc_i):
                sps = psum.tile([128, TCH], f32, tag="acc", bufs=3, name="sps")
                nc.tensor.matmul(
                    sps[:],
                    kt[:, s * 128 : (s + 1) * 128],
                    qt[:, h, tc_i * TCH : (tc_i + 1) * TCH],
                    start=True,
                    stop=True,
                )
                return sps

            rs_all = dram.tile(
                [NTCH, D // 4, TCH], f16, tag="rs_all", name="rs_all"
            )
            for tc_i in range(NTCH):
                at = work.tile([128, M, TCH], f16, tag="at", bufs=2, name="at")

                def flush_head(h, o_sbs):
                    # PE transposes + copies for a finished head (deferred)
                    for tb in range(4):
                        tp = psum.tile([128, 128], f16, tag="tp", bufs=1, name="tp")
                        nc.tensor.transpose(tp[:], o_sbs[tb][:], ident[:])
                        nc.vector.tensor_copy(
                            at[:, h, tb * 128 : (tb + 1) * 128], tp[:]
                        )

                iters = [(h, s) for h in range(M) for s in range(NSB)]
                opks = None
                pending = None  # (h, [o_sb x4]) waiting for transpose
                sps_cur = issue_score(0, 0, tc_i)
                for idx, (h, s) in enumerate(iters):
                    if s == 0:
                        opks = [
                            psum.tile(
                                [128, 129], f32, tag="opk", bufs=4, name=f"opk{i}"
                            )
                            for i in range(4)
                        ]
                    p_sb = work.tile([128, TCH], f16, tag="p", bufs=3, name="p_sb")
                    nc.scalar.activation(p_sb[:], sps_cur[:], Exp)
                    if idx + 1 < len(iters):
                        nh, ns = iters[idx + 1]
                        sps_next = issue_score(nh, ns, tc_i)
                    else:
                        sps_next = None
                    for tb in range(4):
                        nc.tensor.matmul(
                            opks[tb][:, 0:129],
                            p_sb[:, tb * 128 : (tb + 1) * 128],
                            v_sb[:, s, 0:129],
                            start=(s == 0),
                            stop=(s == NSB - 1),
                        )
                    sps_cur = sps_next
                    if s == 1 and pending is not None:
                        flush_head(*pending)
                        pending = None
                    if s == NSB - 1:
                        # normalize on vector now; defer PE transposes one
                        # iteration so the vector deps resolve off-path
                        o_sbs = []
                        for tb in range(4):
                            opk = opks[tb]
                            rcp = work.tile(
                                [128, 1], f32, tag="rcp", bufs=2, name="rcp"
                            )
                            nc.vector.reciprocal(rcp[:], opk[:, 128:129])
                            o_sb = work.tile(
                                [128, 128], f16, tag="osb", bufs=8, name="osb"
                            )
                            nc.vector.tensor_scalar_mul(
                                o_sb[:], opk[:, 0:128], rcp[:]
                            )
                            o_sbs.append(o_sb)
                        pending = (h, o_sbs)
                if pending is not None:
                    flush_head(*pending)
                    pending = None

                # o_proj partial (transposed): partial^T[c, t] for this chunk
                def oproj(partial_tile, cb0, cb1):
                    for cb in range(cb0, cb1):
                        pp = psum.tile(
                            [128, TCH], f32, tag="acc", bufs=3, name="pp"
                        )
                        for h in range(M):
                            nc.tensor.matmul(
                                pp[:],
                                wo_sb[:, h, cb * 128 : (cb + 1) * 128],
                                at[:, h, :],
                                start=(h == 0),
                                stop=(h == M - 1),
                            )
                        po_sb = work.tile(
                            [128, TCH], f16, tag="po", bufs=3, name="po_sb"
                        )
                        nc.vector.tensor_scalar_add(
                            po_sb[:], pp[:], bo4[:, cb : cb + 1]
                        )
                        nc.sync.dma_start(
                            partial_tile[
                                (cb - cb0) * 128 : (cb - cb0 + 1) * 128, :
                            ],
                            po_sb[:],
                        )

                def rs(partial_tile, out_ap):
                    nc.gpsimd.collective_compute(
                        "ReduceScatter",
                        mybir.AluOpType.add,
                        replica_groups=groups,
                        ins=[partial_tile[:]],
                        outs=[out_ap],
                    )

                if tc_i < NTCH - 1:
                    partial = dram.tile(
                        [D, TCH], f16, tag=f"ptl{tc_i}", name=f"partial{tc_i}"
                    )
                    oproj(partial, 0, D // 128)
                    rs(partial, rs_all[tc_i])
                else:
                    # drain earlier chunks to the output while computing;
                    # split the last chunk's RS so only half is exposed
                    for i in range(NTCH - 1):
                        nc.sync.dma_start(outT[i], rs_all[i])
                    pa = dram.tile([D // 2, TCH], f16, tag="ptl3a", name="ptl3a")
                    pb = dram.tile([D // 2, TCH], f16, tag="ptl3b", name="ptl3b")
                    oproj(pa, 0, D // 256)
                    rs(pa, rs_all[tc_i][0 : D // 8, :])
                    oproj(pb, D // 256, D // 128)
                    rs(pb, rs_all[tc_i][D // 8 : D // 4, :])
                    nc.sync.dma_start(outT[tc_i], rs_all[tc_i])

    nc.compile()
    return nc


def _get_nc():
    if "nc" not in _COMPILED:
        _COMPILED["nc"] = _build()
    return _COMPILED["nc"]


def _panel_major(w, nks):
    # [nks*128, C] -> [128, nks*C] with panel index folded into the free dim
    c = w.shape[1]
    return np.ascontiguousarray(
        w.reshape(nks, 128, c).transpose(1, 0, 2).reshape(128, nks * c)
    )


def kernel(x, Wq, bq, Wk, bk, Wv, bv, Wo, bo):
    from concourse.bass_utils import run_bass_kernel_spmd

    x = np.asarray(x, np.float32)
    Wq = np.asarray(Wq, np.float32)
    Wk = np.asarray(Wk, np.float32)
    Wv = np.asarray(Wv, np.float32)
    Wo = np.asarray(Wo, np.float32)
    bq = np.asarray(bq, np.float32)
    bk = np.asarray(bk, np.float32)
    bv = np.asarray(bv, np.float32)
    bo = np.asarray(bo, np.float32)

    nc = _get_nc()

    in_maps = []
    for c in range(N_CORES):
        b, g = c // 4, c % 4
        in_maps.append(
            {
                "xT": np.ascontiguousarray(x[b].T).astype(np.float16),
                "wq": _panel_major(
                    Wq[:, g * GQ : (g + 1) * GQ].astype(np.float16), NKS
                ),
                "wk": _panel_major(
                    Wk[:, g * HD : (g + 1) * HD].astype(np.float16), NKS
                ),
                "wv": _panel_major(
                    Wv[:, g * HD : (g + 1) * HD].astype(np.float16), NKS
                ),
                "wo": _panel_major(
                    Wo[g * GQ : (g + 1) * GQ, :].astype(np.float16), M
                ),
                "bqs": np.ascontiguousarray(
                    (bq[g * GQ : (g + 1) * GQ] * SCALE).reshape(M, 128).T
                ),
                "bks": np.ascontiguousarray(
                    bk[g * HD : (g + 1) * HD].reshape(1, 128).T
                ),
                "bvs": np.ascontiguousarray(
                    bv[g * HD : (g + 1) * HD].reshape(1, 128).T
                ),
                "bo4": np.ascontiguousarray((bo / 4.0).reshape(D // 128, 128).T),
            }
        )

    res = run_bass_kernel_spmd(nc, in_maps, list(range(N_CORES)))
    _COMPILED["last_res"] = res

    out = np.empty((B, T, D), np.float32)
    for c in range(N_CORES):
        b, r = c // 4, c % 4
        ot = res.results[c]["outT"].astype(np.float32)  # [NTCH, 512, TCH]
        for ti in range(NTCH - 1):
            out[b, ti * TCH : (ti + 1) * TCH, r * 512 : (r + 1) * 512] = ot[ti].T
        # last chunk was reduced as two half-RS ops: rows 0:256 hold
        # d=[256r,256r+256), rows 256:512 hold d=[1024+256r, 1024+256r+256)
        t3 = slice((NTCH - 1) * TCH, NTCH * TCH)
        out[b, t3, 256 * r : 256 * r + 256] = ot[NTCH - 1][0:256].T
        out[b, t3, 1024 + 256 * r : 1024 + 256 * r + 256] = ot[NTCH - 1][256:512].T
    return out


# revision 20
# speedup vs baseline: 1.0285x; 1.0285x over previous
"""GroupedQueryAttention on 8 Trainium2 NeuronCores.

Sharding: core c = 4*b + g handles batch b (of 2) and KV group g (of 4),
i.e. 4 query heads (512 q-dims) + one 128-dim K/V head. o_proj is computed
as per-group partials (transposed layout); partials are reduced with a
per-t-chunk fp16 ReduceScatter across the 4 cores of each batch, pipelined
behind the next chunk's compute so only the last chunk's collective is
exposed. Core c ends up with a [512 d-rows x 512 t] quarter of out^T per
chunk; the host reassembles.

All matmuls run in fp16 (1 PE cycle/row) with fp32 PSUM accumulation.
Layouts avoid transposing the big P matrix:
  - projections produce Q^T/K^T directly (lhsT=W tile, rhs=x^T tile),
    processed panel-major in groups of 3 chains so the PE stays busy
    while x^T panels stream in
  - scores are computed as S^T = (K^T).T @ Q^T
  - exp(S^T) = P^T feeds A@V as lhsT directly
  - V carries an extra ones-column so the softmax denominator falls out
    of the A@V matmul for free; normalization is applied to the small
    A@V output rather than to P.
  - the attention inner loop is software-pipelined: the next (h,s) score
    matmul is issued ahead of the current AV matmuls so the PE stays busy
    while the Scalar engine runs exp; the PE transposes of the finished
    head's output are deferred one iteration so their vector deps resolve
    off the critical path.
  - weight loads are single descriptors on the Scalar DMA queue, x^T
    panel loads on the Sync queue.
"""

import math
import sys

import numpy as np

sys.path.insert(0, "/opt/trn_rl_repo")

B = 2
T = 2048
D = 2048
HEADS = 16
GROUPS = 4
HD = 128  # head dim
M = HEADS // GROUPS  # heads per group = 4
GQ = M * HD  # q dims per group = 512
SCALE = 1.0 / math.sqrt(HD)
N_CORES = 8
TCH = 512  # t chunk
NTCH = T // TCH  # 4
NSB = T // 128  # 16 s blocks
NKS = D // 128  # 16 contraction steps for projections

_COMPILED = {}


def _build():
    import concourse.bass as bass
    import concourse.mybir as mybir
    import concourse.tile as tile
    from concourse import bacc
    from concourse.masks import make_identity

    f16 = mybir.dt.float16
    f32 = mybir.dt.float32
    Exp = mybir.ActivationFunctionType.Exp

    nc = bacc.Bacc("TRN2", target_bir_lowering=False, num_devices=N_CORES)

    xT = nc.declare_dram_parameter("xT", [D, T], f16, isOutput=False)
    # weights host-rearranged to partition-major single-DMA layouts
    wq = nc.declare_dram_parameter("wq", [128, NKS * GQ], f16, isOutput=False)
    wk = nc.declare_dram_parameter("wk", [128, NKS * HD], f16, isOutput=False)
    wv = nc.declare_dram_parameter("wv", [128, NKS * HD], f16, isOutput=False)
    wo = nc.declare_dram_parameter("wo", [128, M * D], f16, isOutput=False)
    bqs_d = nc.declare_dram_parameter("bqs", [128, M], f32, isOutput=False)
    bks_d = nc.declare_dram_parameter("bks", [128, 1], f32, isOutput=False)
    bvs_d = nc.declare_dram_parameter("bvs", [128, 1], f32, isOutput=False)
    bo4_d = nc.declare_dram_parameter("bo4", [128, D // 128], f32, isOutput=False)
    # per-chunk ReduceScatter output: core r of the 4-core group gets rows
    # [512r, 512(r+1)) of out^T for each t-chunk
    outT = nc.declare_dram_parameter(
        "outT", [NTCH, D // 4, TCH], f16, isOutput=True
    )

    groups = [[0, 1, 2, 3], [4, 5, 6, 7]]

    with tile.TileContext(nc) as tc:
        with (
            tc.tile_pool(name="const", bufs=1) as const,
            tc.tile_pool(name="work", bufs=2) as work,
            tc.tile_pool(name="psum", bufs=1, space="PSUM") as psum,
            tc.tile_pool(name="dram", bufs=1, space="DRAM") as dram,
        ):
            ident = const.tile([128, 128], f16)
            make_identity(nc, ident)
            bqs = const.tile([128, M], f32)
            bks = const.tile([128, 1], f32)
            bvs = const.tile([128, 1], f32)
            bo4 = const.tile([128, D // 128], f32)

            # x^T panels as independent tiles so the first chain steps only
            # depend on the panels they actually read
            xt = [const.tile([128, T], f16, name=f"xt{i}") for i in range(NKS)]
            wq_sb = const.tile([128, NKS, GQ], f16)
            wk_sb = const.tile([128, NKS, HD], f16)
            wv_sb = const.tile([128, NKS, HD], f16)
            wo_sb = const.tile([128, M, D], f16)
            # weights on the Scalar DMA queue (single descriptors), x^T
            # panels on the Sync queue in consumption order
            # split the x^T panel loads across both DMA queues so early
            # panels finish sooner; wk leads the scalar queue (needed first)
            nc.scalar.dma_start(wk_sb[:, :, :], wk[:])
            nc.scalar.dma_start(wv_sb[:, :, :], wv[:])
            nc.scalar.dma_start(bqs[:], bqs_d[:])
            nc.scalar.dma_start(bks[:], bks_d[:])
            nc.scalar.dma_start(bvs[:], bvs_d[:])
            nc.scalar.dma_start(bo4[:], bo4_d[:])
            for i in range(NKS):
                q = nc.sync if i % 2 == 0 else nc.scalar
                q.dma_start(xt[i][:], xT[i * 128 : (i + 1) * 128, :])

            qt = const.tile([128, M, T], f16)
            kt = const.tile([128, T], f16)
            vt_sb = const.tile([128, T], f16)
            v_sb = const.tile([128, NSB, 132], f16)
            nc.vector.memset(v_sb[:, :, 128:129], 1.0)

            # ---- projections, panel-major in groups of 3 chains ----
            # chain = (weight tile, col-slice, t-chunk, writeback fn)
            def wb_k(acc, tc_i):
                nc.vector.tensor_scalar_add(
                    kt[:, tc_i * TCH : (tc_i + 1) * TCH], acc[:], bks[:, 0:1]
                )

            def wb_v(acc, tc_i):
                nc.vector.tensor_scalar_add(
                    vt_sb[:, tc_i * TCH : (tc_i + 1) * TCH], acc[:], bvs[:, 0:1]
                )

            def wb_q(h):
                def wb(acc, tc_i):
                    nc.vector.tensor_scalar(
                        qt[:, h, tc_i * TCH : (tc_i + 1) * TCH],
                        acc[:],
                        SCALE,
                        bqs[:, h : h + 1],
                        op0=mybir.AluOpType.mult,
                        op1=mybir.AluOpType.add,
                    )

                return wb

            chains = []
            for tc_i in range(NTCH):
                chains.append((wk_sb, 0, HD, tc_i, wb_k))
            for tc_i in range(NTCH):
                chains.append((wv_sb, 0, HD, tc_i, wb_v))
            for tc_i in range(NTCH):
                for h in range(M):
                    chains.append((wq_sb, h * 128, (h + 1) * 128, tc_i, wb_q(h)))

            # big weight loads issued after the K/V chains exist so they
            # don't contend with the early x^T panels for HBM bandwidth
            nc.scalar.dma_start(wq_sb[:, :, :], wq[:])
            nc.scalar.dma_start(wo_sb[:, :, :], wo[:])

            for g0 in range(0, len(chains), 3):
                grp = chains[g0 : g0 + 3]
                accs = [
                    psum.tile([128, TCH], f32, tag="acc", bufs=3, name="acc")
                    for _ in grp
                ]
                for ks in range(NKS):
                    for ci, (wtile, c0, c1, tc_i, _wb) in enumerate(grp):
                        nc.tensor.matmul(
                            accs[ci][:],
                            wtile[:, ks, c0:c1],
                            xt[ks][:, tc_i * TCH : (tc_i + 1) * TCH],
                            start=(ks == 0),
                            stop=(ks == NKS - 1),
                        )
                for ci, (_w, _c0, _c1, tc_i, wb) in enumerate(grp):
                    wb(accs[ci], tc_i)

            # V natural [s, hd] (+ ones col) via PE transpose
            for s in range(NSB):
                tp = psum.tile([128, 128], f16, tag="tp", bufs=1, name="tp")
                nc.tensor.transpose(tp[:], vt_sb[:, s * 128 : (s + 1) * 128], ident[:])
                nc.vector.tensor_copy(v_sb[:, s, 0:128], tp[:])

            # ---- attention + o_proj per t-chunk, RS pipelined behind ----
            def issue_score(h, s, tc_i):
                sps = psum.tile([128, TCH], f32, tag="acc", bufs=3, name="sps")
                nc.tensor.matmul(
                    sps[:],
                    kt[:, s * 128 : (s + 1) * 128],
                    qt[:, h, tc_i * TCH : (tc_i + 1) * TCH],
                    start=True,
                    stop=True,
                )
                return sps

            rs_all = dram.tile(
                [NTCH, D // 4, TCH], f16, tag="rs_all", name="rs_all"
            )
            for tc_i in range(NTCH):
                at = work.tile([128, M, TCH], f16, tag="at", bufs=2, name="at")

                def flush_head(h, o_sbs):
                    # PE transposes + copies for a finished head (deferred)
                    for tb in range(4):
                        tp = psum.tile([128, 128], f16, tag="tp", bufs=1, name="tp")
                        nc.tensor.transpose(tp[:], o_sbs[tb][:], ident[:])
                        nc.vector.tensor_copy(
                            at[:, h, tb * 128 : (tb + 1) * 128], tp[:]
                        )

                iters = [(h, s) for h in range(M) for s in range(NSB)]
                opks = None
                pending = None  # (h, [o_sb x4]) waiting for transpose
                sps_cur = issue_score(0, 0, tc_i)
                for idx, (h, s) in enumerate(iters):
                    if s == 0:
                        opks = [
                            psum.tile(
                                [128, 129], f32, tag="opk", bufs=4, name=f"opk{i}"
                            )
                            for i in range(4)
                        ]
                    p_sb = work.tile([128, TCH], f16, tag="p", bufs=3, name="p_sb")
                    nc.scalar.activation(p_sb[:], sps_cur[:], Exp)
                    if idx + 1 < len(iters):
                        nh, ns = iters[idx + 1]
                        sps_next = issue_score(nh, ns, tc_i)
                    else:
                        sps_next = None
                    for tb in range(4):
                        nc.tensor.matmul(
                            opks[tb][:, 0:129],
                            p_sb[:, tb * 128 : (tb + 1) * 128],
                            v_sb[:, s, 0:129],
                            start=(s == 0),
                            stop=(s == NSB - 1),
                        )
                    sps_cur = sps_next
                    if s == 1 and pending is not None:
                        flush_head(*pending)
                        pending = None
                    if s == NSB - 1:
                        # normalize on vector now; defer PE transposes one
                        # iteration so the vector deps resolve off-path
                        o_sbs = []
                        for tb in range(4):
                            opk = opks[tb]
                            rcp = work.tile(
                                [128, 1], f32, tag="rcp", bufs=2, name="rcp"
                            )
                            nc.vector.reciprocal(rcp[:], opk[:, 128:129])
                            o_sb = work.tile(
                                [128, 128], f16, tag="osb", bufs=8, name="osb"
                            )
                            nc.vector.tensor_scalar_mul(
                                o_sb[:], opk[:, 0:128], rcp[:]
                            )
                            o_sbs.append(o_sb)
                        pending = (h, o_sbs)
                if pending is not None:
                    flush_head(*pending)
                    pending = None

                # o_proj partial (transposed): partial^T[c, t] for this chunk
                def oproj(partial_tile, cb0, cb1):
                    for cb in range(cb0, cb1):
                        pp = psum.tile(
                            [128, TCH], f32, tag="acc", bufs=3, name="pp"
                        )
                        for h in range(M):
                            nc.tensor.matmul(
                                pp[:],
                                wo_sb[:, h, cb * 128 : (cb + 1) * 128],
                                at[:, h, :],
                                start=(h == 0),
                                stop=(h == M - 1),
                            )
                        po_sb = work.tile(
                            [128, TCH], f16, tag="po", bufs=3, name="po_sb"
                        )
                        nc.vector.tensor_scalar_add(
                            po_sb[:], pp[:], bo4[:, cb : cb + 1]
                        )
                        nc.sync.dma_start(
                            partial_tile[
                                (cb - cb0) * 128 : (cb - cb0 + 1) * 128, :
                            ],
                            po_sb[:],
                        )

                def rs(partial_tile, out_ap):
                    nc.gpsimd.collective_compute(
                        "ReduceScatter",
                        mybir.AluOpType.add,
                        replica_groups=groups,
                        ins=[partial_tile[:]],
                        outs=[out_ap],
                    )

                if tc_i < NTCH - 1:
                    partial = dram.tile(
                        [D, TCH], f16, tag=f"ptl{tc_i}", name=f"partial{tc_i}"
                    )
                    oproj(partial, 0, D // 128)
                    rs(partial, rs_all[tc_i])
                else:
                    # drain earlier chunks to the output while computing;
                    # split the last chunk's RS so only half is exposed
                    for i in range(NTCH - 1):
                        nc.sync.dma_start(outT[i], rs_all[i])
                    pa = dram.tile([D // 2, TCH], f16, tag="ptl3a", name="ptl3a")
                    pb = dram.tile([D // 2, TCH], f16, tag="ptl3b", name="ptl3b")
                    oproj(pa, 0, D // 256)
                    rs(pa, rs_all[tc_i][0 : D // 8, :])
                    oproj(pb, D // 256, D // 128)
                    rs(pb, rs_all[tc_i][D // 8 : D // 4, :])
                    nc.sync.dma_start(outT[tc_i], rs_all[tc_i])

    nc.compile()
    return nc


def _get_nc():
    if "nc" not in _COMPILED:
        _COMPILED["nc"] = _build()
    return _COMPILED["nc"]


def _panel_major(w, nks):
    # [nks*128, C] -> [128, nks*C] with panel index folded into the free dim
    c = w.shape[1]
    return np.ascontiguousarray(
        w.reshape(nks, 128, c).transpose(1, 0, 2).reshape(128, nks * c)
    )


def kernel(x, Wq, bq, Wk, bk, Wv, bv, Wo, bo):
    from concourse.bass_utils import run_bass_kernel_spmd

    x = np.asarray(x, np.float32)
    Wq = np.asarray(Wq, np.float32)
    Wk = np.asarray(Wk, np.float32)
    Wv = np.asarray(Wv, np.float32)
    Wo = np.asarray(Wo, np.float32)
    bq = np.asarray(bq, np.float32)
    bk = np.asarray(bk, np.float32)
    bv = np.asarray(bv, np.float32)
    bo = np.asarray(bo, np.float32)

    nc = _get_nc()

    in_maps = []
    for c in range(N_CORES):
        b, g = c // 4, c % 4
        in_maps.append(
            {
                "xT": np.ascontiguousarray(x[b].T).astype(np.float16),
                "wq": _panel_major(
                    Wq[:, g * GQ : (g + 1) * GQ].astype(np.float16), NKS
                ),
                "wk": _panel_major(
                    Wk[:, g * HD : (g + 1) * HD].astype(np.float16), NKS
                ),
                "wv": _panel_major(
                    Wv[:, g * HD : (g + 1) * HD].astype(np.float16), NKS
                ),
                "wo": _panel_major(
                    Wo[g * GQ : (g + 1) * GQ, :].astype(np.float16), M
                ),
                "bqs": np.ascontiguousarray(
                    (bq[g * GQ : (g + 1) * GQ] * SCALE).reshape(M, 128).T
                ),
                "bks": np.ascontiguousarray(
                    bk[g * HD : (g + 1) * HD].reshape(1, 128).T
                ),
                "bvs": np.ascontiguousarray(
                    bv[g * HD : (g + 1) * HD].reshape(1, 128).T
                ),
                "bo4": np.ascontiguousarray((bo / 4.0).reshape(D // 128, 128).T),
            }
        )

    res = run_bass_kernel_spmd(nc, in_maps, list(range(N_CORES)))
    _COMPILED["last_res"] = res

    out = np.empty((B, T, D), np.float32)
    for c in range(N_CORES):
        b, r = c // 4, c % 4
        ot = res.results[c]["outT"].astype(np.float32)  # [NTCH, 512, TCH]
        for ti in range(NTCH - 1):
            out[b, ti * TCH : (ti + 1) * TCH, r * 512 : (r + 1) * 512] = ot[ti].T
        # last chunk was reduced as two half-RS ops: rows 0:256 hold
        # d=[256r,256r+256), rows 256:512 hold d=[1024+256r, 1024+256r+256)
        t3 = slice((NTCH - 1) * TCH, NTCH * TCH)
        out[b, t3, 256 * r : 256 * r + 256] = ot[NTCH - 1][0:256].T
        out[b, t3, 1024 + 256 * r : 1024 + 256 * r + 256] = ot[NTCH - 1][256:512].T
    return out
